# revision 2
# baseline (speedup 1.0000x reference)
"""Self-contained Trainium2 Bass kernel for nn_Encode_64579128262771 (v2).

6-pass shared-weight LSTM encoder (B=128, T=256, H=512):
  pass 0:   lstm(x,  W1,U1,b1, 0,0)
  pass 1-4: lstm(seq,W1,U1,b1, h,c)   (same weights, chained state)
  pass 5:   lstm(seq,W2,U2,b2, h,c) -> returns last h

Sharding: 8-way data-parallel over batch (16 rows/core); the recurrence is
sequential in time and across passes, and cross-core exchange has a ~5us
floor on this toolchain, so each core runs its slice independently.

Design (transposed-gate layout):
  - All recurrent state is stored transposed: h^T, c^T as [128, 4, 16]
    ([hidden%128, hidden//128, batch]) and z^T per step as [128, 256]
    ([gate%128, gate//128 * 16 + batch]).  Elementwise ops then use all 128
    SBUF partitions (8x fewer cycles/step than the natural [16, ...] layout).
  - Recurrent matmul z^T[mt] = sum_k U[k-chunk, mt-chunk]^T-loaded-stationary
    @ h^T[k-chunk]: 64 (LDWEIGHTS + MATMUL N=16) pairs per step, bf16
    weights so fast-weight-load kicks in (~34 ns/tile measured).
  - h^T is produced directly by the last gate multiply (o * tanh(c)) into
    the stage buffer that the next step's matmuls stream as their moving
    operand -- no transposes anywhere in the loop.
  - Gate column order is permuted to [i | f | o | g] blocks; g gets a true
    Tanh, i/f/o a Sigmoid, and the per-gate bias is applied for free in the
    pre-GEMM's PSUM->SBUF copy (scalar-engine Identity activation with a
    per-partition bias vector, possible only in the transposed layout).
  - The pre-GEMM for pass p+1 (zx' = seq_p @ W) is fused into pass p's
    recurrent loop with a one-block lag, reading the 8-step h^T stage
    directly from SBUF (seq never round-trips through DRAM); its matmuls
    fill the tensor engine's dependency-stall slots between steps.
  - zx (per-pass [2048, 4096]) is double-buffered in DRAM as bf16 with a
    one-block pad so the lagged writer never needs a negative index.
  - Everything except PSUM/c/gate-activations is bf16 (rel err ~2e-3,
    tolerance 2e-2); c and the gate outputs stay f32.
"""

import sys

sys.path.insert(0, "/opt/trn_rl_repo")

import numpy as np

import concourse.bass as bass
import concourse.mybir as mybir
from concourse.tile import TileContext
from bass_rust import ScopedClock

F32 = mybir.dt.float32
BF16 = mybir.dt.bfloat16
AF = mybir.ActivationFunctionType
ALU = mybir.AluOpType

# ---------------------------------------------------------------------------
# Toolchain workarounds (same as the ones the previous baseline needed)
# ---------------------------------------------------------------------------

_SPLIT_CTR = [0]


def split_multiwaits(nc):
    """This container's walrus encodes at most ONE semaphore wait per
    instruction; hoist extra waits onto single-wait NoOps just before the
    instruction (same engine, so semantics are identical)."""
    n_fixed = 0
    for f in nc.m.functions:
        for bb in f.blocks:
            insts = bb.instructions
            if not any(
                i.sync_info is not None
                and i.sync_info.on_wait
                and len(i.sync_info.on_wait) > 1
                for i in insts
            ):
                continue
            newl = []
            for inst in insts:
                si = inst.sync_info
                if si is not None and si.on_wait and len(si.on_wait) > 1:
                    waits = list(si.on_wait)
                    for w in waits[:-1]:
                        _SPLIT_CTR[0] += 1
                        nop = mybir.InstNoOp(
                            name=f"I-mwsplit-{_SPLIT_CTR[0]}", ins=[], outs=[]
                        )
                        nop.engine = inst.engine
                        nop.sync_info = mybir.SyncInfo(on_wait=[w], on_update=[])
                        newl.append(nop)
                    inst.sync_info = mybir.SyncInfo(
                        on_wait=[waits[-1]], on_update=list(si.on_update)
                    )
                    n_fixed += 1
                newl.append(inst)
            bb.instructions = newl
    return n_fixed


def _patched_drain_and_barrier(self, tick_clock, wait_clock):
    """Same as TileContext._drain_and_barrier but emits the final global-clock
    waits as individual single-wait SP instructions (1-wait ISA limit)."""
    nc = self.nc
    probe = mybir.InstNoOp(name="I-probe-drain", ins=[], outs=[])
    probe.engine = mybir.EngineType.SP
    wait_clock.add_sem_waits(probe, ScopedClock({None: tick_clock.global_clock}))
    si = probe.sync_info
    waits = list(si.on_wait) if si is not None else []
    id2handle = {h.num: h for h in self.sems.allocated().values()}
    for w in waits:
        h = id2handle.get(w.id)
        assert h is not None, f"no handle for sem id {w.id} ({w.ant_name})"
        nc.sync.wait_ge(h, w.wait_value)
    nc.sync.drain()
    nc.all_engine_barrier()
    popped = nc._tile_sem_poison_stack.pop()
    assert popped is self._sem_poison
    nc.clear_and_free_semaphores(list(self.sems.allocated().values()))
    nc.all_engine_barrier()


TileContext._drain_and_barrier = _patched_drain_and_barrier


def install_ntff_hook():
    """Register the NTFF profiling hook that trn_boot skipped because the
    image's antenv package lacks axon_hooks (needed for trace=True)."""
    import types

    if "antenv.axon_hooks" in sys.modules:
        return
    mod = types.ModuleType("antenv.axon_hooks")
    mod._hook = None

    def _set(h):
        mod._hook = h

    def _get():
        return mod._hook

    mod.set_axon_ntff_profile_hook = _set
    mod.get_axon_ntff_profile_hook = _get
    import antenv

    sys.modules["antenv.axon_hooks"] = mod
    antenv.axon_hooks = mod
    try:
        from trn_agent_boot.trn_boot import _ntff_profile_via_ctypes

        _set(_ntff_profile_via_ctypes("/opt/axon/libaxon_pjrt.so"))
    except Exception:
        pass
    import concourse.bass_utils as bu

    bu.upload_artifacts = lambda tmpdir: tmpdir


# ---------------------------------------------------------------------------
# Constants / layout
# ---------------------------------------------------------------------------

H = 512
G4 = 4 * H  # 2048
T = 256
Bc = 16  # batch rows per core
NMT = 16  # gate m-tiles of 128
NK = 4  # hidden k-chunks of 128
NBLK = T // 8  # 32 blocks of 8 steps
NZBLK = NBLK + 1  # zx DRAM blocks incl one leading pad block

# Column storage is GROUP-MAJOR: two k-pair groups (hidden chunks {0,1} and
# {2,3}); within a group, gate blocks [i | f | o | g], each 2x128 cols
# (kk within the group). All of one group's state lives in its own tiles so
# the two groups' dependency chains are fully independent (Tile tracks
# dependencies per tile, not per element).


# ---------------------------------------------------------------------------
# Kernel builder
# ---------------------------------------------------------------------------


def build_kernel(n_passes=6, with_a=True, with_a0=True):
    nc = bass.Bass()

    xbT = nc.declare_dram_parameter("xbT", [128, NK, T * Bc], BF16, isOutput=False)
    wb1 = nc.declare_dram_parameter("wb1", [128, NK, G4], BF16, isOutput=False)
    wb2 = nc.declare_dram_parameter("wb2", [128, NK, G4], BF16, isOutput=False)
    ur1 = nc.declare_dram_parameter("ur1", [128, NK, G4], BF16, isOutput=False)
    ur2 = nc.declare_dram_parameter("ur2", [128, NK, G4], BF16, isOutput=False)
    bt1 = nc.declare_dram_parameter("bt1", [128, NMT], F32, isOutput=False)
    bt2 = nc.declare_dram_parameter("bt2", [128, NMT], F32, isOutput=False)
    ht_o = nc.declare_dram_parameter("ht_o", [128, NK, Bc], BF16, isOutput=True)

    with TileContext(nc) as tc:
        dram = tc.alloc_tile_pool(name="dram", bufs=1, space="DRAM")
        zxA = dram.tile([NZBLK, 128, NMT, 128], BF16)
        zxB = dram.tile([NZBLK, 128, NMT, 128], BF16)

        wpool = tc.alloc_tile_pool(name="w", bufs=1)
        wb1S = wpool.tile([128, NK, G4], BF16)
        wb2S = wpool.tile([128, NK, G4], BF16)
        ur1S = wpool.tile([128, NK, G4], BF16)
        ur2S = wpool.tile([128, NK, G4], BF16)
        bt1S = wpool.tile([128, NMT], F32)
        bt2S = wpool.tile([128, NMT], F32)
        # h^T stages split by k-pair half so a step's group-0 hN write does
        # not false-depend against group-1 readers
        stA01 = wpool.tile([128, 2, 8, Bc], BF16)
        stA23 = wpool.tile([128, 2, 8, Bc], BF16)
        stB01 = wpool.tile([128, 2, 8, Bc], BF16)
        stB23 = wpool.tile([128, 2, 8, Bc], BF16)
        cT0 = wpool.tile([128, 2, Bc], F32)
        cT1 = wpool.tile([128, 2, Bc], F32)

        zpsum = tc.alloc_tile_pool(name="zp", bufs=2, space="PSUM")
        apsum = tc.alloc_tile_pool(name="ap", bufs=4, space="PSUM")
        ztile = tc.alloc_tile_pool(name="zt", bufs=2)
        gpool = tc.alloc_tile_pool(name="g", bufs=2)
        zxsp = tc.alloc_tile_pool(name="zxs", bufs=2)
        zxnp = tc.alloc_tile_pool(name="zxn", bufs=2)
        xsp = tc.alloc_tile_pool(name="xs", bufs=2)

        # one-time loads / init
        nc.sync.dma_start(out=wb1S[:], in_=wb1[:])
        nc.sync.dma_start(out=wb2S[:], in_=wb2[:])
        nc.sync.dma_start(out=ur1S[:], in_=ur1[:])
        nc.sync.dma_start(out=ur2S[:], in_=ur2[:])
        nc.sync.dma_start(out=bt1S[:], in_=bt1[:])
        nc.sync.dma_start(out=bt2S[:], in_=bt2[:])
        for t_ in (stA01, stA23, stB01, stB23, cT0, cT1):
            nc.vector.memset(t_[:], 0.0)

        def a_group(mt, wbS, btS, st01, st23, zxn, on_act):
            """One pre-GEMM m-tile: zxn[:, mt, :] = (seq8 @ W)[:, mt] + b.
            The bias-add + PSUM->SBUF copy alternates between DVE and the
            Scalar engine to balance their queues."""
            ap = apsum.tile([128, 128], F32, tag="ap")
            for k in range(NK):
                st = st01 if k < 2 else st23
                nc.tensor.matmul(
                    ap[:],
                    wbS[:, k, mt * 128 : (mt + 1) * 128],
                    st[:, k % 2, :, :],
                    start=(k == 0),
                    stop=(k == NK - 1),
                )
            del on_act
            nc.vector.tensor_scalar(
                zxn[:, mt, :], ap[:], btS[:, mt : mt + 1], None, ALU.add
            )

        def b_step(urS, zxs, u, hs01, hs23, hd01, hd23, a_work):
            """One recurrent step. hs01/hs23: [128, 2, Bc] h^T sources
            (chunks {0,1} / {2,3}); hd01/hd23: destinations.

            Matmul rounds: group-0 m-tiles contract chunks {0,1} first (they
            only need hd01 of the previous step), then chunks {2,3}, then all
            of group 1; each group has its own PSUM/z/sig/... tiles so its
            chain starts as soon as its own accumulation closes."""
            P0 = zpsum.tile([128, 128], F32, tag="zp0", name="P0")
            P1 = zpsum.tile([128, 128], F32, tag="zp1", name="P1")
            zT0 = ztile.tile([128, 128], F32, tag="zT0", name="zT0")
            zT1 = ztile.tile([128, 128], F32, tag="zT1", name="zT1")
            sg0 = gpool.tile([128, 96], F32, tag="sg0", name="sg0")
            sg1 = gpool.tile([128, 96], F32, tag="sg1", name="sg1")
            tg0 = gpool.tile([128, 32], F32, tag="tg0", name="tg0")
            tg1 = gpool.tile([128, 32], F32, tag="tg1", name="tg1")
            m1_0 = gpool.tile([128, 32], F32, tag="m10", name="m1_0")
            m1_1 = gpool.tile([128, 32], F32, tag="m11", name="m1_1")
            m2_0 = gpool.tile([128, 32], F32, tag="m20", name="m2_0")
            m2_1 = gpool.tile([128, 32], F32, tag="m21", name="m2_1")
            th0 = gpool.tile([128, 32], F32, tag="th0", name="th0")
            th1 = gpool.tile([128, 32], F32, tag="th1", name="th1")

            def mm(P, loc, kk, hs, first):
                # start=True clears has_written for the WHOLE PSUM bank, so
                # with interleaved per-loc accumulation groups only the very
                # first matmul into each P tile may carry it; the others'
                # first writes land on cleared has_written bits and
                # overwrite, later ones accumulate.
                nc.tensor.matmul(
                    P[:, loc * 16 : (loc + 1) * 16],
                    urS[:, kk, (loc if P is P0 else loc + 8) * 128 :][:, :128],
                    hs[:, kk % 2, :],
                    start=first,
                    stop=(kk == NK - 1),
                    skip_group_check=True,
                )

            # round 1: group 0, contraction chunks 0-1
            for kk in (0, 1):
                for loc in range(8):
                    mm(P0, loc, kk, hs01, kk == 0 and loc == 0)
            # round 2: group 0, contraction chunks 2-3 (closes P0)
            for kk in (2, 3):
                for loc in range(8):
                    mm(P0, loc, kk, hs23, False)

            def chain(P, zT, sg, tg, m1, m2, th, cTg, zxsl, hdst):
                k2 = lambda ap: ap.rearrange("p (k b) -> p k b", k=2)
                nc.vector.tensor_tensor(
                    zT[:].rearrange("p (a b) -> p a b", a=8),
                    P[:].rearrange("p (a b) -> p a b", a=8),
                    zxsl, ALU.add,
                )
                nc.scalar.activation(sg[:], zT[:, 0:96], AF.Sigmoid)
                nc.scalar.activation(tg[:], zT[:, 96:128], AF.Tanh)
                nc.vector.tensor_tensor(k2(m1[:]), k2(sg[:, 32:64]), cTg[:], ALU.mult)
                nc.vector.tensor_tensor(m2[:], sg[:, 0:32], tg[:], ALU.mult)
                nc.vector.tensor_tensor(cTg[:], k2(m1[:]), k2(m2[:]), ALU.add)
                nc.scalar.activation(k2(th[:]), cTg[:], AF.Tanh)
                nc.vector.tensor_tensor(hdst, k2(sg[:, 64:96]), k2(th[:]), ALU.mult)

            zxv = zxs[:, :, u * 16 : (u + 1) * 16]
            chain(P0, zT0, sg0, tg0, m1_0, m2_0, th0, cT0, zxv[:, 0:8, :], hd01)

            # round 3: group 1, all contraction chunks (closes P1)
            for kk in range(NK):
                for loc in range(8):
                    mm(P1, loc, kk, hs01 if kk < 2 else hs23,
                       kk == 0 and loc == 0)
            chain(P1, zT1, sg1, tg1, m1_1, m2_1, th1, cT1, zxv[:, 8:16, :], hd23)

            for w in a_work:
                w()

        # ---- pass 0 pre-GEMM from x (prologue) ----
        def a0_body(it):
            xs = xsp.tile([128, NK, 128], BF16, tag="xs")
            nc.scalar.dma_start(out=xs[:], in_=xbT[:, :, bass.ds(it * 128, 128)])
            zxn = zxnp.tile([128, NMT, 128], BF16, tag="zxn")
            for mt in range(NMT):
                ap = apsum.tile([128, 128], F32, tag="ap")
                for k in range(NK):
                    nc.tensor.matmul(
                        ap[:],
                        wb1S[:, k, mt * 128 : (mt + 1) * 128],
                        xs[:, k, :],
                        start=(k == 0),
                        stop=(k == NK - 1),
                    )
                nc.scalar.activation(
                    zxn[:, mt, :], ap[:], AF.Identity, bias=bt1S[:, mt : mt + 1]
                )
            nc.sync.dma_start(
                out=zxA[bass.ds(it + 1, 1)].rearrange("a p m c -> p (a m) c"),
                in_=zxn[:],
            )

        if with_a0:
            tc.For_i_unrolled(0, NBLK, 1, a0_body, max_unroll=1)

        # ---- passes 0..5 ----
        for p in range(n_passes):
            urS = ur1S if p < 5 else ur2S
            zx_cur = zxA if p % 2 == 0 else zxB
            zx_nxt = zxB if p % 2 == 0 else zxA
            has_a = (p < 5) and with_a
            wbS = (wb1S if p < 4 else wb2S) if has_a else None
            btS = (bt1S if p < 4 else bt2S) if has_a else None

            def pass_body(j, urS=urS, zx_cur=zx_cur, zx_nxt=zx_nxt,
                          has_a=has_a, wbS=wbS, btS=btS):
                # one combined read for both half-blocks (register budget:
                # dynamic-offset DMAs consume engine registers per site)
                zxs2 = zxsp.tile([128, 2, NMT, 128], BF16, tag="zxs")
                nc.sync.dma_start(
                    out=zxs2[:],
                    in_=zx_cur[bass.ds(j * 2 + 1, 2)].rearrange(
                        "a p m c -> p a m c"
                    ),
                )
                zxs_a = zxs2[:, 0]
                zxn2 = (
                    zxnp.tile([128, 2, NMT, 128], BF16, tag="zxn", name="zxn2")
                    if has_a
                    else None
                )
                zxn_b = zxn2[:, 0] if has_a else None
                for u in range(8):
                    hs01 = stB01[:, :, 7, :] if u == 0 else stA01[:, :, u - 1, :]
                    hs23 = stB23[:, :, 7, :] if u == 0 else stA23[:, :, u - 1, :]
                    aw = []
                    if has_a:
                        for i_, mt in enumerate((2 * u, 2 * u + 1)):
                            aw.append(
                                lambda mt=mt, i_=i_: a_group(
                                    mt, wbS, btS, stB01, stB23, zxn_b, i_ == 0
                                )
                            )
                    b_step(urS, zxs_a, u, hs01, hs23,
                           stA01[:, :, u, :], stA23[:, :, u, :], aw)

                # second half: steps 8-15 -> stB; A for this iter's stA
                zxs_b = zxs2[:, 1]
                zxn_a = zxn2[:, 1] if has_a else None
                for u in range(8):
                    hs01 = stA01[:, :, 7, :] if u == 0 else stB01[:, :, u - 1, :]
                    hs23 = stA23[:, :, 7, :] if u == 0 else stB23[:, :, u - 1, :]
                    aw = []
                    if has_a:
                        for i_, mt in enumerate((2 * u, 2 * u + 1)):
                            aw.append(
                                lambda mt=mt, i_=i_: a_group(
                                    mt, wbS, btS, stA01, stA23, zxn_a, i_ == 0
                                )
                            )
                    b_step(urS, zxs_b, u, hs01, hs23,
                           stB01[:, :, u, :], stB23[:, :, u, :], aw)
                if has_a:
                    nc.scalar.dma_start(
                        out=zx_nxt[bass.ds(j * 2, 2)].rearrange(
                            "a p m c -> p a m c"
                        ),
                        in_=zxn2[:],
                    )

            tc.For_i_unrolled(0, NBLK // 2, 1, pass_body, max_unroll=1)

            # epilogue: pre-GEMM for the last block (stB of final iteration)
            if has_a:
                zxn_e = zxnp.tile([128, NMT, 128], BF16, tag="zxn")
                for mt in range(NMT):
                    a_group(mt, wbS, btS, stB01, stB23, zxn_e, mt % 2 == 0)
                nc.sync.dma_start(
                    out=zx_nxt[NBLK], in_=zxn_e[:]
                )

        nc.sync.dma_start(out=ht_o[:, 0:2, :], in_=stB01[:, :, 7, :])
        nc.sync.dma_start(out=ht_o[:, 2:4, :], in_=stB23[:, :, 7, :])

        for pool in (xsp, zxnp, zxsp, gpool, ztile, apsum, zpsum, wpool, dram):
            pool.release()

    split_multiwaits(nc)
    return nc


# ---------------------------------------------------------------------------
# Host-side prep
# ---------------------------------------------------------------------------

# column permutation: group-major storage. new block (grp, gate, j) of 128
# cols <- original cols gate_orig*512 + 128*(2*grp+j) + m, with new gate
# order [i f o g] <-> original [i f g o].
_GATE_ORIG = [0, 1, 3, 2]
_PERM = np.concatenate(
    [
        np.arange(
            _GATE_ORIG[gate] * 512 + 128 * (2 * grp + j),
            _GATE_ORIG[gate] * 512 + 128 * (2 * grp + j) + 128,
        )
        for grp in range(2)
        for gate in range(4)
        for j in range(2)
    ]
)


def _prep_w(W):
    """W [512, 2048] -> [128, 4, 2048] bf16-ready (k-chunk rows)."""
    Wp = W[:, _PERM]
    return np.ascontiguousarray(Wp.reshape(NK, 128, G4).transpose(1, 0, 2))


def _prep_b(b):
    bp = b[_PERM]
    return np.ascontiguousarray(bp.reshape(NMT, 128).T)  # [128, 16]


def _prep_x(x_slice):
    """x [16, 256, 512] -> xbT [128, 4, 4096]: xbT[p,k,t*16+b] = x[b,t,128k+p]."""
    xt = x_slice.transpose(2, 1, 0)  # [512, 256, 16]
    return np.ascontiguousarray(
        xt.reshape(NK, 128, T * Bc).transpose(1, 0, 2)
    )


def make_inputs_for_core(x_core, W1, U1, b1, W2, U2, b2):
    import ml_dtypes

    bf = lambda a: np.asarray(a, np.float32).astype(ml_dtypes.bfloat16)
    return {
        "xbT": bf(_prep_x(np.asarray(x_core, np.float32))),
        "wb1": bf(_prep_w(np.asarray(W1, np.float32))),
        "wb2": bf(_prep_w(np.asarray(W2, np.float32))),
        "ur1": bf(_prep_w(np.asarray(U1, np.float32))),
        "ur2": bf(_prep_w(np.asarray(U2, np.float32))),
        "bt1": _prep_b(np.asarray(b1, np.float32)),
        "bt2": _prep_b(np.asarray(b2, np.float32)),
    }


def assemble_h(ht_res):
    """ht_res [128, 4, 16] -> h [16, 512]: h[b, 128k+p] = ht[p, k, b]."""
    ht = np.asarray(ht_res).astype(np.float32)
    return np.ascontiguousarray(ht.transpose(2, 1, 0).reshape(Bc, H))


_BUILT = {}


def _build():
    if "nc" not in _BUILT:
        _BUILT["nc"] = build_kernel()
    return _BUILT["nc"]


def kernel(x, W1, U1, b1, W2, U2, b2):
    """Full inputs in, full output out. x [128,256,512] f32 -> h [128,512]."""
    from concourse.bass_utils import run_bass_kernel_spmd

    x = np.asarray(x, dtype=np.float32)
    nc = _build()
    in_maps = [
        make_inputs_for_core(
            x[c * Bc : (c + 1) * Bc], W1, U1, b1, W2, U2, b2
        )
        for c in range(8)
    ]
    res = run_bass_kernel_spmd(nc, in_maps, list(range(8)))
    parts = [assemble_h(res.results[i]["ht_o"]) for i in range(8)]
    return np.concatenate(parts, axis=0)
